# revision 1
# baseline (speedup 1.0000x reference)
"""Trainium2 Bass kernel for CrossAttention (B=4, SQ=SKV=2048, 16 heads).

Sharding: 8 cores = (batch b in 0..3) x (head-half hh in 0..1).
Each core computes 8 heads (512 of the 1024 d_att columns) for one batch,
producing a partial pre-bias output projection; partials for the two
head-halves of each batch are summed on the host, along with the constant
correction row bo + bv @ Wo (the value-bias passes through softmax as a
constant because softmax rows sum to 1).

Device dataflow (all activations kept feature-on-partition, "T layout"):
  host supplies x[b].T and y[b].T so every matmul contracts along the
  partition dim with natural-layout DMAs; scores are computed transposed
  (ST[skv, sq] = K @ Q^T) so the exp'd scores feed the P@V matmul directly
  as the moving operand with V as the stationary operand, and the softmax
  denominators come for free from a ones-column appended to V.
"""

import numpy as np
import ml_dtypes

import concourse.bass as bass
from concourse import mybir
from concourse.bass_utils import run_bass_kernel_spmd
from concourse.tile import TileContext, ScopedClock

BF16 = mybir.dt.bfloat16
F32 = mybir.dt.float32
NP_BF16 = ml_dtypes.bfloat16

B, SQ, SKV = 4, 2048, 2048
D_EMBED, D_CROSS, D_ATT, N_HEADS = 1024, 768, 1024, 16
D_HEAD = 64
DHC = 512          # d_att columns per core (8 heads)
NHC = 8            # heads per core
SCALE = 1.0 / 8.0  # 1/sqrt(D_HEAD)

KE = D_EMBED // 128   # 8  k-tiles for x projections
KC = D_CROSS // 128   # 6  k-tiles for y projections
KO = DHC // 128       # 4  k-tiles for out projection
NSQ = SQ // 512       # 4  sq blocks of 512
NKV = SKV // 128      # 16 skv tiles of 128


def _patched_drain_and_barrier(self, tick_clock, wait_clock):
    # walrus CoreV2/V3 codegen caps sync waits at 1 per CTRL instruction; the
    # stock kernel-tail drain carries one wait per active proc.  Collect the
    # waits on a probe NOP and spread them across a chain of NOPs.
    probe = self.nc.sync.nop()
    wait_clock.add_sem_waits(probe.ins, ScopedClock({None: tick_clock.global_clock}))
    waits = list(probe.ins.sync_info.on_wait)
    probe.ins.sync_info.on_wait = waits[:1]
    rest = waits[1:]
    si_cls = type(probe.ins.sync_info)
    while rest:
        n = self.nc.sync.nop()
        if n.ins.sync_info is None:
            n.ins.sync_info = si_cls(on_wait=rest[:1], on_update=[])
        else:
            n.ins.sync_info.on_wait = rest[:1]
        rest = rest[1:]
    self.nc.sync.drain()
    self.nc.all_engine_barrier()
    popped = self.nc._tile_sem_poison_stack.pop()
    assert popped is self._sem_poison
    self.nc.clear_and_free_semaphores(list(self.sems.allocated().values()))
    self.nc.all_engine_barrier()


TileContext._drain_and_barrier = _patched_drain_and_barrier


def _split_excess_waits(nc, max_waits=1):
    """This walrus build caps sync waits per instruction (1 for CTRL ops).
    Hoist excess sem waits onto preceding same-engine NOPs: the engine
    stalls on the NOPs first, so the original instruction still executes
    only after every wait holds (ge-waits are monotonic, so early
    evaluation is safe)."""
    for fn in nc.m.functions:
        for bb in fn.blocks:
            new_list = []
            changed = False
            for inst in bb.instructions:
                si = getattr(inst, "sync_info", None)
                if si is not None and si.on_wait and len(si.on_wait) > max_waits:
                    waits = list(si.on_wait)
                    extra, keep = waits[:-max_waits], waits[-max_waits:]
                    for j, w in enumerate(extra):
                        new_list.append(mybir.InstNoOp(
                            name=f"{inst.name}-w{j}",
                            sync_info=mybir.SyncInfo(on_wait=[w], on_update=[]),
                            engine=inst.engine,
                            bass_nofuse=True,
                        ))
                    si.on_wait = keep
                    changed = True
                new_list.append(inst)
            if changed:
                bb.instructions[:] = new_list


def _build_program():
    nc = bass.Bass()
    xT = nc.declare_dram_parameter("xT", [D_EMBED, SQ], BF16, isOutput=False)
    yT = nc.declare_dram_parameter("yT", [D_CROSS, SKV], BF16, isOutput=False)
    wq = nc.declare_dram_parameter("wq", [D_EMBED, DHC], BF16, isOutput=False)
    wk = nc.declare_dram_parameter("wk", [D_CROSS, DHC], BF16, isOutput=False)
    wv = nc.declare_dram_parameter("wv", [D_CROSS, DHC], BF16, isOutput=False)
    wo = nc.declare_dram_parameter("wo", [DHC, D_EMBED], BF16, isOutput=False)
    bq = nc.declare_dram_parameter("bq", [DHC], F32, isOutput=False)  # pre-scaled
    bk = nc.declare_dram_parameter("bk", [DHC], F32, isOutput=False)
    outT = nc.declare_dram_parameter("outT", [D_EMBED, SQ], F32, isOutput=True)

    with TileContext(nc) as tc:
        with tc.tile_pool(name="res", bufs=1) as res:
            # --- resident SBUF state ---
            xT_sb = [res.tile([128, SQ], BF16, tag=f"xT{k}", name=f"xT{k}") for k in range(KE)]
            yT_sb = [res.tile([128, SKV], BF16, tag=f"yT{k}", name=f"yT{k}") for k in range(KC)]
            wq_sb = [res.tile([128, DHC], BF16, tag=f"wq{k}", name=f"wq{k}") for k in range(KE)]
            wk_sb = [res.tile([128, DHC], BF16, tag=f"wk{k}", name=f"wk{k}") for k in range(KC)]
            wv_sb = [res.tile([128, DHC], BF16, tag=f"wv{k}", name=f"wv{k}") for k in range(KC)]
            wo_sb = [res.tile([128, D_EMBED], BF16, tag=f"wo{k}", name=f"wo{k}") for k in range(KO)]
            kt_sb = [res.tile([128, SKV], BF16, tag=f"kt{m}", name=f"kt{m}") for m in range(KO)]
            # qt/ot split per sq-half so filler writes never alias attention reads
            qt_sb = [[res.tile([128, 1024], BF16, tag=f"qt{m}_{j}", name=f"qt{m}_{j}")
                      for j in range(2)] for m in range(KO)]
            ot_sb = [[res.tile([128, 1024], BF16, tag=f"ot{m}_{j}", name=f"ot{m}_{j}")
                      for j in range(2)] for m in range(KO)]
            v_sb = [res.tile([128, NHC, D_HEAD + 1], BF16, tag=f"v{i}", name=f"v{i}") for i in range(NKV)]
            bq_sb = res.tile([128, KO], F32, tag="bq", name="bq")
            bk_sb = res.tile([128, KO], F32, tag="bk", name="bk")
            ones_sb = res.tile([1, D_HEAD], F32, tag="ones", name="ones")
            nc.vector.memset(ones_sb, 1.0)

            for k in range(KC):
                nc.sync.dma_start(out=yT_sb[k], in_=yT[k * 128:(k + 1) * 128, :])
                nc.sync.dma_start(out=wv_sb[k], in_=wv[k * 128:(k + 1) * 128, :])
                nc.sync.dma_start(out=wk_sb[k], in_=wk[k * 128:(k + 1) * 128, :])
            for k in range(KE):
                nc.sync.dma_start(out=xT_sb[k], in_=xT[k * 128:(k + 1) * 128, :])
                nc.sync.dma_start(out=wq_sb[k], in_=wq[k * 128:(k + 1) * 128, :])
            for k in range(KO):
                nc.sync.dma_start(out=wo_sb[k], in_=wo[k * 128:(k + 1) * 128, :])
            nc.sync.dma_start(out=bq_sb, in_=bq.rearrange("(m p) -> p m", p=128))
            nc.sync.dma_start(out=bk_sb, in_=bk.rearrange("(m p) -> p m", p=128))

            def emit_qt_chain(psum_pool, m, n):
                # n indexes 512-blocks of sq; writes qt_sb[m][n // 2]
                ps = psum_pool.tile([128, 512], F32, tag="fill", name="qtp")
                for k in range(KE):
                    nc.tensor.matmul(
                        out=ps,
                        lhsT=wq_sb[k][:, m * 128:(m + 1) * 128],
                        rhs=xT_sb[k][:, n * 512:(n + 1) * 512],
                        start=(k == 0), stop=(k == KE - 1),
                    )
                    yield
                nc.vector.tensor_scalar(
                    out=qt_sb[m][n // 2][:, (n % 2) * 512:(n % 2) * 512 + 512], in0=ps,
                    scalar1=SCALE, scalar2=bq_sb[:, m:m + 1],
                    op0=mybir.AluOpType.mult, op1=mybir.AluOpType.add,
                )
                yield

            def emit_outproj_chain(psum_pool, out_pool, mo, n):
                ps = psum_pool.tile([128, 512], F32, tag="fill", name="opp")
                for k in range(KO):
                    nc.tensor.matmul(
                        out=ps,
                        lhsT=wo_sb[k][:, mo * 128:(mo + 1) * 128],
                        rhs=ot_sb[k][n // 2][:, (n % 2) * 512:(n % 2) * 512 + 512],
                        start=(k == 0), stop=(k == KO - 1),
                    )
                    yield
                ob = out_pool.tile([128, 512], F32, tag="ob", name="ob")
                nc.vector.tensor_copy(out=ob, in_=ps)
                nc.sync.dma_start(
                    out=outT[mo * 128:(mo + 1) * 128, n * 512:(n + 1) * 512],
                    in_=ob,
                )
                yield

            # --- phase A: V proj, K proj, Q proj (first sq half) ---
            with tc.tile_pool(name="psA", bufs=3, space="PSUM") as psA, \
                 tc.tile_pool(name="osbA", bufs=2) as osbA:
                for i in range(NKV):
                    vp = psA.tile([128, DHC], F32, tag="ps", name="vp")
                    for k in range(KC):
                        nc.tensor.matmul(
                            out=vp,
                            lhsT=yT_sb[k][:, i * 128:(i + 1) * 128],
                            rhs=wv_sb[k],
                            start=(k == 0), stop=(k == KC - 1),
                        )
                    nc.vector.tensor_copy(
                        out=v_sb[i][:, :, 0:D_HEAD],
                        in_=vp.rearrange("p (h d) -> p h d", h=NHC),
                    )
                    nc.vector.memset(v_sb[i][:, :, D_HEAD:D_HEAD + 1], 1.0)
                for m in range(KO):
                    for n in range(NSQ):
                        kp = psA.tile([128, 512], F32, tag="ps", name="kp")
                        for k in range(KC):
                            nc.tensor.matmul(
                                out=kp,
                                lhsT=wk_sb[k][:, m * 128:(m + 1) * 128],
                                rhs=yT_sb[k][:, n * 512:(n + 1) * 512],
                                start=(k == 0), stop=(k == KC - 1),
                            )
                        nc.vector.tensor_scalar_add(
                            out=kt_sb[m][:, n * 512:(n + 1) * 512], in0=kp,
                            scalar1=bk_sb[:, m:m + 1],
                        )
                for m in range(KO):
                    for n in range(2):  # first sq half only
                        for _ in emit_qt_chain(psA, m, n):
                            pass

            # --- phase B: attention with interleaved PE filler work ---
            # Head pairs (2t, 2t+1) issue their score matmuls into disjoint
            # 64-row PE array groups (tile_position from base_partition), so
            # the two K=64 matmuls run concurrently; one [128,1024] PSUM/exp
            # block covers both heads for one 512-col sq block.
            with tc.tile_pool(name="ps_st", bufs=2, space="PSUM") as ps_st, \
                 tc.tile_pool(name="ps_ot", bufs=2, space="PSUM") as ps_ot, \
                 tc.tile_pool(name="ps_fill", bufs=2, space="PSUM") as ps_fill, \
                 tc.tile_pool(name="work", bufs=3) as work, \
                 tc.tile_pool(name="norm", bufs=4) as norm, \
                 tc.tile_pool(name="osb", bufs=3) as osb:

                # jj-major order: each quarter of the step space completes one
                # 512-col block of ot_sb for every head, unblocking that
                # block's output projection as filler work for the next quarter
                groups = [(j2, hp, jj) for j2 in range(2) for jj in range(2)
                          for hp in range(KO)]

                def qt_gen():
                    for m in range(KO):
                        for n in range(2):
                            yield from emit_qt_chain(ps_fill, m, n + 2)

                def op_gen(n):
                    def g():
                        for mo in range(D_EMBED // 128):
                            yield from emit_outproj_chain(ps_fill, osb, mo, n)
                    return g()

                # (generator, earliest step its inputs are ready + slack)
                filler_plan = [(qt_gen(), 0), (op_gen(0), 72), (op_gen(1), 132),
                               (op_gen(2), 196), (op_gen(3), 1 << 30)]

                _DONE = object()

                def pull_filler(step):
                    for gen, earliest in filler_plan:
                        if step < earliest:
                            return
                        if next(gen, _DONE) is not _DONE:
                            return

                def do_st(g, i):
                    j2, hp, jj = g
                    st = ps_st.tile([128, 1024], F32, tag="st", name="st")
                    for hs in range(2):
                        po = hs * 64
                        nc.tensor.matmul(
                            out=st[:, hs * 512:(hs + 1) * 512],
                            lhsT=kt_sb[hp][po:po + 64, i * 128:(i + 1) * 128],
                            rhs=qt_sb[hp][j2][po:po + 64, jj * 512:(jj + 1) * 512],
                            start=True, stop=True,
                        )
                    return st

                def do_exp(st):
                    pt = work.tile([128, 1024], BF16, tag="pt", name="pt")
                    nc.scalar.activation(
                        out=pt, in_=st, func=mybir.ActivationFunctionType.Exp,
                    )
                    return pt

                ots_cur = {}

                def do_pv(g, i, pt):
                    j2, hp, jj = g
                    if i == 0:
                        ots_cur[g] = [ps_ot.tile([D_HEAD + 1, 512], F32,
                                                 tag="ot", name="ot")
                                      for _ in range(2)]
                    for hs in range(2):
                        nc.tensor.matmul(
                            out=ots_cur[g][hs],
                            lhsT=v_sb[i][:, 2 * hp + hs, :],
                            rhs=pt[:, hs * 512:(hs + 1) * 512],
                            start=(i == 0), stop=(i == NKV - 1),
                        )

                def do_norm_stage1(g):
                    parts = []
                    for hs in range(2):
                        otf = norm.tile([D_HEAD + 1, 512], F32, tag="otf", name="otf")
                        nc.vector.tensor_copy(out=otf, in_=ots_cur[g][hs])
                        den0 = norm.tile([1, 512], F32, tag="den0", name="den0")
                        nc.sync.dma_start(out=den0, in_=otf[D_HEAD:D_HEAD + 1, :])
                        rec0 = norm.tile([1, 512], F32, tag="rec0", name="rec0")
                        nc.vector.reciprocal(out=rec0, in_=den0)
                        parts.append((hs, otf, rec0))
                    del ots_cur[g]
                    return parts

                def do_norm_stage2(g, parts):
                    j2, hp, jj = g
                    sq0 = jj * 512
                    for hs, otf, rec0 in parts:
                        recb = ps_fill.tile([D_HEAD, 512], F32, tag="fill", name="recb")
                        nc.tensor.matmul(out=recb, lhsT=ones_sb, rhs=rec0,
                                         start=True, stop=True)
                        if hs == 0:
                            nc.vector.tensor_mul(
                                out=ot_sb[hp][j2][0:D_HEAD, sq0:sq0 + 512],
                                in0=otf[0:D_HEAD, :], in1=recb,
                            )
                        else:
                            tmp = norm.tile([D_HEAD, 512], BF16, tag="otmp", name="otmp")
                            nc.vector.tensor_mul(out=tmp, in0=otf[0:D_HEAD, :], in1=recb)
                            nc.sync.dma_start(
                                out=ot_sb[hp][j2][D_HEAD:128, sq0:sq0 + 512], in_=tmp,
                            )

                # one flat software pipeline across every (group, i) step: the
                # PE stream per step is ST(cur) -> filler -> PV(prev) so it
                # never head-of-line blocks on ACT's exp, including across
                # group boundaries
                steps = [(g, i) for g in groups for i in range(NKV)]
                prev = None
                norm_q = []  # (due_step, group, stage1 parts)
                for step_idx, (g, i) in enumerate(steps):
                    st = do_st(g, i)
                    pull_filler(step_idx)
                    while norm_q and norm_q[0][0] <= step_idx:
                        _, ng, parts = norm_q.pop(0)
                        do_norm_stage2(ng, parts)
                    if prev is not None:
                        pg, pi, ppt = prev
                        do_pv(pg, pi, ppt)
                        if pi == NKV - 1:
                            norm_q.append((step_idx + 3, pg, do_norm_stage1(pg)))
                    prev = (g, i, do_exp(st))
                pg, pi, ppt = prev
                do_pv(pg, pi, ppt)
                for _, ng, parts in norm_q:
                    do_norm_stage2(ng, parts)
                do_norm_stage2(pg, do_norm_stage1(pg))

                # drain remaining filler work (the last 512-col block's output
                # projection runs here once its ot tiles land)
                for gen, _earliest in filler_plan:
                    for _ in gen:
                        pass

    _split_excess_waits(nc)
    return nc


_NC = None


def _get_nc():
    global _NC
    if _NC is None:
        _NC = _build_program()
    return _NC


def _run(inputs, trace=False):
    x = np.asarray(inputs["x"], dtype=np.float32)
    y = np.asarray(inputs["y"], dtype=np.float32)
    Wq = np.asarray(inputs["Wq"], dtype=np.float32)
    bq = np.asarray(inputs["bq"], dtype=np.float32)
    Wk = np.asarray(inputs["Wk"], dtype=np.float32)
    bk = np.asarray(inputs["bk"], dtype=np.float32)
    Wv = np.asarray(inputs["Wv"], dtype=np.float32)
    bv = np.asarray(inputs["bv"], dtype=np.float32)
    Wo = np.asarray(inputs["Wo"], dtype=np.float32)
    bo = np.asarray(inputs["bo"], dtype=np.float32)

    in_maps = []
    for c in range(8):
        b, hh = c // 2, c % 2
        h0 = hh * DHC
        in_maps.append({
            "xT": np.ascontiguousarray(x[b].T).astype(NP_BF16),
            "yT": np.ascontiguousarray(y[b].T).astype(NP_BF16),
            "wq": np.ascontiguousarray(Wq[:, h0:h0 + DHC]).astype(NP_BF16),
            "wk": np.ascontiguousarray(Wk[:, h0:h0 + DHC]).astype(NP_BF16),
            "wv": np.ascontiguousarray(Wv[:, h0:h0 + DHC]).astype(NP_BF16),
            "wo": np.ascontiguousarray(Wo[h0:h0 + DHC, :]).astype(NP_BF16),
            "bq": np.ascontiguousarray(bq[h0:h0 + DHC] * SCALE).astype(np.float32),
            "bk": np.ascontiguousarray(bk[h0:h0 + DHC]).astype(np.float32),
        })

    nc = _get_nc()
    res = run_bass_kernel_spmd(nc, in_maps, list(range(8)), trace=trace)

    corr = bo + bv.astype(np.float64) @ Wo.astype(np.float64)  # constant row
    out = np.empty((B, SQ, D_EMBED), dtype=np.float32)
    for b in range(B):
        acc = res.results[2 * b]["outT"].astype(np.float32) + \
              res.results[2 * b + 1]["outT"].astype(np.float32)
        out[b] = acc.T + corr.astype(np.float32)
    return out, res


def kernel(**inputs):
    out, _ = _run(inputs, trace=False)
    return out



# revision 4
# speedup vs baseline: 1.5495x; 1.5495x over previous
"""Trainium2 Bass kernel for CrossAttention (B=4, SQ=SKV=2048, 16 heads).

Sharding: 8 cores = (batch b in 0..3) x (head-half hh in 0..1).
Each core computes 8 heads (512 of the 1024 d_att columns) for one batch,
producing a partial pre-bias output projection; partials for the two
head-halves of each batch are summed on the host, along with the constant
correction row bo + bv @ Wo (the value-bias passes through softmax as a
constant because softmax rows sum to 1).

Device dataflow (all activations kept feature-on-partition, "T layout"):
  host supplies x[b].T and y[b].T so every matmul contracts along the
  partition dim with natural-layout DMAs; scores are computed transposed
  (ST[skv, sq] = K @ Q^T) so the exp'd scores feed the P@V matmul directly
  as the moving operand with V as the stationary operand, and the softmax
  denominators come for free from a ones-column appended to V.
"""

import numpy as np
import ml_dtypes

import concourse.bass as bass
from concourse import mybir
from concourse.bass_utils import run_bass_kernel_spmd
from concourse.tile import TileContext, ScopedClock

BF16 = mybir.dt.bfloat16
F32 = mybir.dt.float32
NP_BF16 = ml_dtypes.bfloat16

B, SQ, SKV = 4, 2048, 2048
D_EMBED, D_CROSS, D_ATT, N_HEADS = 1024, 768, 1024, 16
D_HEAD = 64
DHC = 512          # d_att columns per core (8 heads)
NHC = 8            # heads per core
SCALE = 1.0 / 8.0  # 1/sqrt(D_HEAD)

KE = D_EMBED // 128   # 8  k-tiles for x projections
KC = D_CROSS // 128   # 6  k-tiles for y projections
KO = DHC // 128       # 4  k-tiles for out projection
NSQ = SQ // 512       # 4  sq blocks of 512
NKV = SKV // 128      # 16 skv tiles of 128


def _patched_drain_and_barrier(self, tick_clock, wait_clock):
    # walrus CoreV2/V3 codegen caps sync waits at 1 per CTRL instruction; the
    # stock kernel-tail drain carries one wait per active proc.  Collect the
    # waits on a probe NOP and spread them across a chain of NOPs.
    probe = self.nc.sync.nop()
    wait_clock.add_sem_waits(probe.ins, ScopedClock({None: tick_clock.global_clock}))
    waits = list(probe.ins.sync_info.on_wait)
    probe.ins.sync_info.on_wait = waits[:1]
    rest = waits[1:]
    si_cls = type(probe.ins.sync_info)
    while rest:
        n = self.nc.sync.nop()
        if n.ins.sync_info is None:
            n.ins.sync_info = si_cls(on_wait=rest[:1], on_update=[])
        else:
            n.ins.sync_info.on_wait = rest[:1]
        rest = rest[1:]
    self.nc.sync.drain()
    self.nc.all_engine_barrier()
    popped = self.nc._tile_sem_poison_stack.pop()
    assert popped is self._sem_poison
    self.nc.clear_and_free_semaphores(list(self.sems.allocated().values()))
    self.nc.all_engine_barrier()


TileContext._drain_and_barrier = _patched_drain_and_barrier


def _split_excess_waits(nc, max_waits=1):
    """This walrus build caps sync waits per instruction (1 for CTRL ops).
    Hoist excess sem waits onto preceding same-engine NOPs: the engine
    stalls on the NOPs first, so the original instruction still executes
    only after every wait holds (ge-waits are monotonic, so early
    evaluation is safe)."""
    for fn in nc.m.functions:
        for bb in fn.blocks:
            new_list = []
            changed = False
            for inst in bb.instructions:
                si = getattr(inst, "sync_info", None)
                if si is not None and si.on_wait and len(si.on_wait) > max_waits:
                    waits = list(si.on_wait)
                    extra, keep = waits[:-max_waits], waits[-max_waits:]
                    for j, w in enumerate(extra):
                        new_list.append(mybir.InstNoOp(
                            name=f"{inst.name}-w{j}",
                            sync_info=mybir.SyncInfo(on_wait=[w], on_update=[]),
                            engine=inst.engine,
                            bass_nofuse=True,
                        ))
                    si.on_wait = keep
                    changed = True
                new_list.append(inst)
            if changed:
                bb.instructions[:] = new_list


def _build_program():
    nc = bass.Bass()
    xT = nc.declare_dram_parameter("xT", [D_EMBED, SQ], BF16, isOutput=False)
    yT = nc.declare_dram_parameter("yT", [D_CROSS, SKV], BF16, isOutput=False)
    wq = nc.declare_dram_parameter("wq", [D_EMBED, DHC], BF16, isOutput=False)
    wk = nc.declare_dram_parameter("wk", [D_CROSS, DHC], BF16, isOutput=False)
    wv = nc.declare_dram_parameter("wv", [D_CROSS, DHC], BF16, isOutput=False)
    wo = nc.declare_dram_parameter("wo", [DHC, D_EMBED], BF16, isOutput=False)
    bq = nc.declare_dram_parameter("bq", [DHC], F32, isOutput=False)  # pre-scaled
    bk = nc.declare_dram_parameter("bk", [DHC], F32, isOutput=False)
    outT = nc.declare_dram_parameter("outT", [D_EMBED, SQ], F32, isOutput=True)

    with TileContext(nc) as tc:
        with tc.tile_pool(name="res", bufs=1) as res:
            # --- resident SBUF state ---
            xT_sb = [res.tile([128, SQ], BF16, tag=f"xT{k}", name=f"xT{k}") for k in range(KE)]
            yT_sb = [res.tile([128, SKV], BF16, tag=f"yT{k}", name=f"yT{k}") for k in range(KC)]
            wq_sb = [res.tile([128, DHC], BF16, tag=f"wq{k}", name=f"wq{k}") for k in range(KE)]
            wk_sb = [res.tile([128, DHC], BF16, tag=f"wk{k}", name=f"wk{k}") for k in range(KC)]
            wv_sb = [res.tile([128, DHC], BF16, tag=f"wv{k}", name=f"wv{k}") for k in range(KC)]
            wo_sb = [res.tile([128, D_EMBED], BF16, tag=f"wo{k}", name=f"wo{k}") for k in range(KO)]
            kt_sb = [res.tile([128, SKV], BF16, tag=f"kt{m}", name=f"kt{m}") for m in range(KO)]
            # qt/ot split per sq-half so filler writes never alias attention reads
            qt_sb = [[res.tile([128, 1024], BF16, tag=f"qt{m}_{j}", name=f"qt{m}_{j}")
                      for j in range(2)] for m in range(KO)]
            ot_sb = [[res.tile([128, 1024], BF16, tag=f"ot{m}_{j}", name=f"ot{m}_{j}")
                      for j in range(2)] for m in range(KO)]
            v_sb = [res.tile([128, NHC, D_HEAD + 1], BF16, tag=f"v{i}", name=f"v{i}") for i in range(NKV)]
            bq_sb = res.tile([128, KO], F32, tag="bq", name="bq")
            bk_sb = res.tile([128, KO], F32, tag="bk", name="bk")
            ones_sb = res.tile([1, D_HEAD], BF16, tag="ones", name="ones")
            nc.vector.memset(ones_sb, 1.0)

            for k in range(KC):
                nc.sync.dma_start(out=yT_sb[k], in_=yT[k * 128:(k + 1) * 128, :])
                nc.sync.dma_start(out=wv_sb[k], in_=wv[k * 128:(k + 1) * 128, :])
                nc.sync.dma_start(out=wk_sb[k], in_=wk[k * 128:(k + 1) * 128, :])
            for k in range(KE):
                nc.sync.dma_start(out=xT_sb[k], in_=xT[k * 128:(k + 1) * 128, :])
                nc.sync.dma_start(out=wq_sb[k], in_=wq[k * 128:(k + 1) * 128, :])
            for k in range(KO):
                nc.sync.dma_start(out=wo_sb[k], in_=wo[k * 128:(k + 1) * 128, :])
            nc.sync.dma_start(out=bq_sb, in_=bq.rearrange("(m p) -> p m", p=128))
            nc.sync.dma_start(out=bk_sb, in_=bk.rearrange("(m p) -> p m", p=128))

            def emit_qt_chain(psum_pool, m, n):
                # n indexes 512-blocks of sq; writes qt_sb[m][n // 2]
                ps = psum_pool.tile([128, 512], F32, tag="fill", name="qtp")
                for k in range(KE):
                    nc.tensor.matmul(
                        out=ps,
                        lhsT=wq_sb[k][:, m * 128:(m + 1) * 128],
                        rhs=xT_sb[k][:, n * 512:(n + 1) * 512],
                        start=(k == 0), stop=(k == KE - 1),
                    )
                    yield
                nc.vector.tensor_scalar(
                    out=qt_sb[m][n // 2][:, (n % 2) * 512:(n % 2) * 512 + 512], in0=ps,
                    scalar1=SCALE, scalar2=bq_sb[:, m:m + 1],
                    op0=mybir.AluOpType.mult, op1=mybir.AluOpType.add,
                )
                yield

            def emit_outproj_chain(psum_pool, out_pool, mo, n):
                ps = psum_pool.tile([128, 512], F32, tag="fill", name="opp")
                for k in range(KO):
                    nc.tensor.matmul(
                        out=ps,
                        lhsT=wo_sb[k][:, mo * 128:(mo + 1) * 128],
                        rhs=ot_sb[k][n // 2][:, (n % 2) * 512:(n % 2) * 512 + 512],
                        start=(k == 0), stop=(k == KO - 1),
                    )
                    yield
                ob = out_pool.tile([128, 512], F32, tag="ob", name="ob")
                nc.vector.tensor_copy(out=ob, in_=ps)
                nc.sync.dma_start(
                    out=outT[mo * 128:(mo + 1) * 128, n * 512:(n + 1) * 512],
                    in_=ob,
                )
                yield

            # --- phase A: V proj, K proj, Q proj (first sq half) ---
            with tc.tile_pool(name="psA", bufs=3, space="PSUM") as psA, \
                 tc.tile_pool(name="osbA", bufs=2) as osbA:
                for i in range(NKV):
                    vp = psA.tile([128, DHC], F32, tag="ps", name="vp")
                    for k in range(KC):
                        nc.tensor.matmul(
                            out=vp,
                            lhsT=yT_sb[k][:, i * 128:(i + 1) * 128],
                            rhs=wv_sb[k],
                            start=(k == 0), stop=(k == KC - 1),
                        )
                    nc.vector.tensor_copy(
                        out=v_sb[i][:, :, 0:D_HEAD],
                        in_=vp.rearrange("p (h d) -> p h d", h=NHC),
                    )
                    nc.vector.memset(v_sb[i][:, :, D_HEAD:D_HEAD + 1], 1.0)
                for m in range(KO):
                    for n in range(NSQ):
                        kp = psA.tile([128, 512], F32, tag="ps", name="kp")
                        for k in range(KC):
                            nc.tensor.matmul(
                                out=kp,
                                lhsT=wk_sb[k][:, m * 128:(m + 1) * 128],
                                rhs=yT_sb[k][:, n * 512:(n + 1) * 512],
                                start=(k == 0), stop=(k == KC - 1),
                            )
                        nc.vector.tensor_scalar_add(
                            out=kt_sb[m][:, n * 512:(n + 1) * 512], in0=kp,
                            scalar1=bk_sb[:, m:m + 1],
                        )
                for m in range(KO):
                    for n in range(2):  # first sq half only
                        for _ in emit_qt_chain(psA, m, n):
                            pass

            # --- phase B: attention with interleaved PE filler work ---
            # Head pairs (2t, 2t+1) issue their score matmuls into disjoint
            # 64-row PE array groups (tile_position from base_partition), so
            # the two K=64 matmuls run concurrently; one [128,1024] PSUM/exp
            # block covers both heads for one 512-col sq block.
            with tc.tile_pool(name="ps_st", bufs=2, space="PSUM") as ps_st, \
                 tc.tile_pool(name="ps_ot", bufs=2, space="PSUM") as ps_ot, \
                 tc.tile_pool(name="ps_fill", bufs=2, space="PSUM") as ps_fill, \
                 tc.tile_pool(name="work", bufs=3) as work, \
                 tc.tile_pool(name="norm", bufs=4) as norm, \
                 tc.tile_pool(name="osb", bufs=3) as osb:

                # jj-major order: each quarter of the step space completes one
                # 512-col block of ot_sb for every head, unblocking that
                # block's output projection as filler work for the next quarter
                groups = [(j2, hp, jj) for j2 in range(2) for jj in range(2)
                          for hp in range(KO)]

                def qt_gen():
                    for m in range(KO):
                        for n in range(2):
                            yield from emit_qt_chain(ps_fill, m, n + 2)

                def op_gen(n):
                    def g():
                        for mo in range(D_EMBED // 128):
                            yield from emit_outproj_chain(ps_fill, osb, mo, n)
                    return g()

                # (generator, earliest step its inputs are ready + slack)
                filler_plan = [(qt_gen(), 0), (op_gen(0), 72), (op_gen(1), 132),
                               (op_gen(2), 196), (op_gen(3), 1 << 30)]

                _DONE = object()

                def pull_filler(step):
                    for gen, earliest in filler_plan:
                        if step < earliest:
                            return
                        if next(gen, _DONE) is not _DONE:
                            return

                def do_st(g, i):
                    j2, hp, jj = g
                    st = ps_st.tile([128, 1024], F32, tag="st", name="st")
                    for hs in range(2):
                        po = hs * 64
                        nc.tensor.matmul(
                            out=st[:, hs * 512:(hs + 1) * 512],
                            lhsT=kt_sb[hp][po:po + 64, i * 128:(i + 1) * 128],
                            rhs=qt_sb[hp][j2][po:po + 64, jj * 512:(jj + 1) * 512],
                            start=True, stop=True,
                        )
                    return st

                def do_exp(st):
                    pt = work.tile([128, 1024], BF16, tag="pt", name="pt")
                    nc.scalar.activation(
                        out=pt, in_=st, func=mybir.ActivationFunctionType.Exp,
                    )
                    return pt

                ots_cur = {}

                def do_pv(g, i, pt):
                    j2, hp, jj = g
                    if i == 0:
                        ots_cur[g] = [ps_ot.tile([D_HEAD + 1, 512], F32,
                                                 tag="ot", name="ot")
                                      for _ in range(2)]
                    for hs in range(2):
                        nc.tensor.matmul(
                            out=ots_cur[g][hs],
                            lhsT=v_sb[i][:, 2 * hp + hs, :],
                            rhs=pt[:, hs * 512:(hs + 1) * 512],
                            start=(i == 0), stop=(i == NKV - 1),
                        )

                def do_norm_stage1(g):
                    # Denominators live in one SBUF row ([1,512]); DVE work on a
                    # single partition runs ~8 cyc/elem on one lane, so spread
                    # them across 128 partitions via DMA before the reciprocal,
                    # then return them to row layout (bf16) for the broadcast
                    # matmul's moving operand.
                    otfs = []
                    den_sp = norm.tile([128, 8], F32, tag="den_sp", name="den_sp")
                    for hs in range(2):
                        otf = norm.tile([D_HEAD + 1, 512], F32, tag="otf", name="otf")
                        nc.vector.tensor_copy(out=otf, in_=ots_cur[g][hs])
                        nc.sync.dma_start(out=den_sp[:, hs * 4:hs * 4 + 4],
                                          in_=otf[D_HEAD:D_HEAD + 1, :])
                        otfs.append(otf)
                    rec_sp = norm.tile([128, 8], F32, tag="rec_sp", name="rec_sp")
                    nc.vector.reciprocal(out=rec_sp, in_=den_sp)
                    rec_bf = norm.tile([128, 8], BF16, tag="rec_bf", name="rec_bf")
                    nc.vector.tensor_copy(out=rec_bf, in_=rec_sp)
                    parts = []
                    for hs in range(2):
                        rec_row = norm.tile([1, 512], BF16, tag="rec_row", name="rec_row")
                        nc.sync.dma_start(out=rec_row,
                                          in_=rec_bf[:, hs * 4:hs * 4 + 4])
                        parts.append((hs, otfs[hs], rec_row))
                    del ots_cur[g]
                    return parts

                def do_norm_stage2(g, parts):
                    j2, hp, jj = g
                    sq0 = jj * 512
                    for hs, otf, rec0 in parts:
                        recb = ps_fill.tile([D_HEAD, 512], F32, tag="fill", name="recb")
                        nc.tensor.matmul(out=recb, lhsT=ones_sb, rhs=rec0,
                                         start=True, stop=True)
                        if hs == 0:
                            nc.vector.tensor_mul(
                                out=ot_sb[hp][j2][0:D_HEAD, sq0:sq0 + 512],
                                in0=otf[0:D_HEAD, :], in1=recb,
                            )
                        else:
                            tmp = norm.tile([D_HEAD, 512], BF16, tag="otmp", name="otmp")
                            nc.vector.tensor_mul(out=tmp, in0=otf[0:D_HEAD, :], in1=recb)
                            nc.sync.dma_start(
                                out=ot_sb[hp][j2][D_HEAD:128, sq0:sq0 + 512], in_=tmp,
                            )

                # one flat software pipeline across every (group, i) step: the
                # PE stream per step is ST(cur) -> filler -> PV(prev) so it
                # never head-of-line blocks on ACT's exp, including across
                # group boundaries
                steps = [(g, i) for g in groups for i in range(NKV)]
                prev = None
                norm_q = []  # (due_step, group, stage1 parts)
                for step_idx, (g, i) in enumerate(steps):
                    st = do_st(g, i)
                    pull_filler(step_idx)
                    while norm_q and norm_q[0][0] <= step_idx:
                        _, ng, parts = norm_q.pop(0)
                        do_norm_stage2(ng, parts)
                    if prev is not None:
                        pg, pi, ppt = prev
                        do_pv(pg, pi, ppt)
                        if pi == NKV - 1:
                            norm_q.append((step_idx + 5, pg, do_norm_stage1(pg)))
                    prev = (g, i, do_exp(st))
                pg, pi, ppt = prev
                do_pv(pg, pi, ppt)
                for _, ng, parts in norm_q:
                    do_norm_stage2(ng, parts)
                do_norm_stage2(pg, do_norm_stage1(pg))

                # drain remaining filler work (the last 512-col block's output
                # projection runs here once its ot tiles land)
                for gen, _earliest in filler_plan:
                    for _ in gen:
                        pass

    _split_excess_waits(nc)
    return nc


_NC = None


def _get_nc():
    global _NC
    if _NC is None:
        _NC = _build_program()
    return _NC


def _run(inputs, trace=False):
    x = np.asarray(inputs["x"], dtype=np.float32)
    y = np.asarray(inputs["y"], dtype=np.float32)
    Wq = np.asarray(inputs["Wq"], dtype=np.float32)
    bq = np.asarray(inputs["bq"], dtype=np.float32)
    Wk = np.asarray(inputs["Wk"], dtype=np.float32)
    bk = np.asarray(inputs["bk"], dtype=np.float32)
    Wv = np.asarray(inputs["Wv"], dtype=np.float32)
    bv = np.asarray(inputs["bv"], dtype=np.float32)
    Wo = np.asarray(inputs["Wo"], dtype=np.float32)
    bo = np.asarray(inputs["bo"], dtype=np.float32)

    in_maps = []
    for c in range(8):
        b, hh = c // 2, c % 2
        h0 = hh * DHC
        in_maps.append({
            "xT": np.ascontiguousarray(x[b].T).astype(NP_BF16),
            "yT": np.ascontiguousarray(y[b].T).astype(NP_BF16),
            "wq": np.ascontiguousarray(Wq[:, h0:h0 + DHC]).astype(NP_BF16),
            "wk": np.ascontiguousarray(Wk[:, h0:h0 + DHC]).astype(NP_BF16),
            "wv": np.ascontiguousarray(Wv[:, h0:h0 + DHC]).astype(NP_BF16),
            "wo": np.ascontiguousarray(Wo[h0:h0 + DHC, :]).astype(NP_BF16),
            "bq": np.ascontiguousarray(bq[h0:h0 + DHC] * SCALE).astype(np.float32),
            "bk": np.ascontiguousarray(bk[h0:h0 + DHC]).astype(np.float32),
        })

    nc = _get_nc()
    res = run_bass_kernel_spmd(nc, in_maps, list(range(8)), trace=trace)

    corr = bo + bv.astype(np.float64) @ Wo.astype(np.float64)  # constant row
    out = np.empty((B, SQ, D_EMBED), dtype=np.float32)
    for b in range(B):
        acc = res.results[2 * b]["outT"].astype(np.float32) + \
              res.results[2 * b + 1]["outT"].astype(np.float32)
        out[b] = acc.T + corr.astype(np.float32)
    return out, res


def kernel(**inputs):
    out, _ = _run(inputs, trace=False)
    return out

